# revision 1
# baseline (speedup 1.0000x reference)
"""Multi-head attention (B=4, N=2048, D=512, H=8, Dh=64) on 8 trn2 cores.

Sharding: core c handles batch b = c//2 and head-group hg = c%2 (4 heads).
Each core computes its batch's attention output for its 4 heads plus the
partial output projection (w_out columns for those heads); the host sums the
two head-group partials per batch.

On-device layout is transposed: the core receives x[b].T ([D, N]) so all
matmul contractions run over the partition dim without any on-device
transposes.  Scores are computed as S^T (keys on partitions, queries free),
softmax exp runs on the scalar engine straight out of PSUM, and the
probability@V matmul consumes S^T directly with a ones-column appended to V
to produce the softmax denominators for free.  Matmuls run in float32r
(full-rate fp32 PE mode).
"""

import sys

for p in ("/opt/trn_rl_repo", "/root/.axon_site/_ro/trn_rl_repo"):
    if p not in sys.path:
        sys.path.insert(0, p)

from contextlib import ExitStack

import numpy as np

import concourse.bass as bass
import concourse.mybir as mybir
import concourse.tile as tile
from concourse import bacc
from concourse.bass_utils import run_bass_kernel_spmd

F32 = mybir.dt.float32
F32R = mybir.dt.float32r
AF = mybir.ActivationFunctionType

N_CORES = 8
B, N, D = 4, 2048, 512
HEADS = 8
DH = 64
SCALE = DH**-0.5
HPC = 4  # heads per core
P = 128
NDT = D // P  # 4 d-tiles
NJT = N // P  # 16 j-tiles
IB = 512  # i-block
NIB = N // IB  # 4 i-blocks
UNIT = 3  # score psum slices ([128,512]) per exp instruction

N_REPS = 1  # replications of the whole body inside one NEFF (for timing)


def build_program(n_reps: int = N_REPS):
    nc = bacc.Bacc("TRN2", target_bir_lowering=False, debug=False,
                   num_devices=N_CORES)
    xT = nc.dram_tensor("xT", [D, N], F32R, kind="ExternalInput").ap()
    wqk = nc.dram_tensor("wqk", [D, 2 * HPC * DH], F32R, kind="ExternalInput").ap()
    wv = nc.dram_tensor("wv", [D, HPC * DH], F32R, kind="ExternalInput").ap()
    wo = nc.dram_tensor("wo", [HPC * DH, D], F32R, kind="ExternalInput").ap()
    bias = nc.dram_tensor("bias", [D, 1], F32, kind="ExternalInput").ap()
    yT = nc.dram_tensor("yT", [D, N], F32, kind="ExternalOutput").ap()
    # DRAM scratch for softmax reciprocal rows (bounce for partition bcast)
    rden = nc.dram_tensor("rden", [2 * NIB, 2 * IB], F32).ap()

    with tile.TileContext(nc) as tc, ExitStack() as ctx:
        sb = ctx.enter_context(tc.tile_pool(name="sb", bufs=1))
        if n_reps > 1:
            ctx.enter_context(tc.For_i(0, n_reps, 1))

        for _rep in range(1):
            # preload the exp activation table while the input DMAs run
            warm = sb.tile([1, 16], F32, tag="warm", bufs=1)
            nc.vector.memset(warm, 0.0)
            nc.scalar.activation(warm, warm, AF.Exp, scale=1.0)
            # ---------------- phase 1: load + QKV projection ----------------
            wqk_sb = []
            wv_sb = []
            wo_sb = []
            bias_sb = []
            for dt in range(NDT):
                t = sb.tile([P, 2 * HPC * DH], F32R, tag="wqk", bufs=NDT)
                nc.sync.dma_start(out=t, in_=wqk[dt * P:(dt + 1) * P, :])
                wqk_sb.append(t)
                t = sb.tile([P, HPC * DH], F32R, tag="wv", bufs=NDT)
                nc.sync.dma_start(out=t, in_=wv[dt * P:(dt + 1) * P, :])
                wv_sb.append(t)
                t = sb.tile([P, 1], F32, tag="bias", bufs=NDT)
                nc.sync.dma_start(out=t, in_=bias[dt * P:(dt + 1) * P, :])
                bias_sb.append(t)
            for h in range(HPC):
                t = sb.tile([DH, D], F32R, tag="wo", bufs=HPC)
                nc.sync.dma_start(out=t, in_=wo[h * DH:(h + 1) * DH, :])
                wo_sb.append(t)

            xt_sb = []
            for dt in range(NDT):
                t = sb.tile([P, N], F32R, tag="big", bufs=8)
                nc.sync.dma_start(out=t, in_=xT[dt * P:(dt + 1) * P, :])
                xt_sb.append(t)

            # QT/KT: [128, N] tiles; rows 0:64 even head of pair, 64:128 odd.
            # et: 0 = Q pair0, 1 = Q pair1, 2 = K pair0, 3 = K pair1
            qkt_sb = []
            with tc.tile_pool(name="ps1", bufs=1, space="PSUM") as ps1:
                for et in range(4):
                    t = sb.tile([P, N], F32R, tag="qkt", bufs=4)
                    qkt_sb.append(t)
                    for nb in range(NIB):
                        pq = ps1.tile([P, IB], F32, tag="qk", bufs=4)
                        for dt in range(NDT):
                            nc.tensor.matmul(
                                pq,
                                lhsT=wqk_sb[dt][:, et * P:(et + 1) * P],
                                rhs=xt_sb[dt][:, nb * IB:(nb + 1) * IB],
                                start=(dt == 0), stop=(dt == NDT - 1),
                            )
                        nc.vector.tensor_copy(t[:, nb * IB:(nb + 1) * IB], pq)

                # V natural [n, e] with a ones column per head: [128, 4*65]
                v_sb = []
                for nt in range(NJT):
                    t = sb.tile([P, HPC * (DH + 1)], F32R, tag="v", bufs=NJT)
                    v_sb.append(t)
                    pv = ps1.tile([P, HPC * DH], F32, tag="v", bufs=2)
                    for dt in range(NDT):
                        nc.tensor.matmul(
                            pv,
                            lhsT=xt_sb[dt][:, nt * P:(nt + 1) * P],
                            rhs=wv_sb[dt],
                            start=(dt == 0), stop=(dt == NDT - 1),
                        )
                    nc.vector.tensor_copy(
                        t.rearrange("p (h c) -> p h c", c=DH + 1)[:, :, 0:DH],
                        pv.rearrange("p (h c) -> p h c", c=DH),
                    )
                    nc.vector.memset(
                        t.bitcast(F32).rearrange(
                            "p (h c) -> p h c", c=DH + 1)[:, :, DH:DH + 1],
                        1.0,
                    )

            # ---------------- phase 2: attention ----------------
            # O^T per head: [64, N] tiles (partition base 0), unnormalized.
            ot_sb = []
            for h in range(HPC):
                ot_t = sb.tile([DH, N], F32R, tag="ot", bufs=HPC)
                ot_sb.append(ot_t)

            slices = [(jt, par) for jt in range(NJT) for par in range(2)]
            units = [slices[i:i + UNIT] for i in range(0, len(slices), UNIT)]

            with tc.tile_pool(name="ps2", bufs=1, space="PSUM") as ps2:
                for pair in range(2):
                    for ib in range(NIB):
                        pv_ps0 = ps2.tile([DH + 1, IB], F32, tag="pv", bufs=2)
                        pv_ps1 = ps2.tile([DH + 1, IB], F32, tag="pv", bufs=2)
                        pv_ps = [pv_ps0, pv_ps1]
                        for unit in units:
                            su = ps2.tile([P, len(unit) * IB], F32, tag="s",
                                          bufs=2)
                            for k, (jt, par) in enumerate(unit):
                                nc.tensor.matmul(
                                    su[:, k * IB:(k + 1) * IB],
                                    lhsT=qkt_sb[2 + pair][
                                        par * DH:(par + 1) * DH,
                                        jt * P:(jt + 1) * P],
                                    rhs=qkt_sb[pair][
                                        par * DH:(par + 1) * DH,
                                        ib * IB:(ib + 1) * IB],
                                    start=True, stop=True,
                                )
                            es = sb.tile([P, len(unit) * IB], F32R, tag="big",
                                         bufs=8)
                            nc.scalar.activation(es, su, AF.Exp, scale=SCALE)
                            for k, (jt, par) in enumerate(unit):
                                h = 2 * pair + par
                                nc.tensor.matmul(
                                    pv_ps[par],
                                    lhsT=v_sb[jt][
                                        :, h * (DH + 1):(h + 1) * (DH + 1)],
                                    rhs=es[:, k * IB:(k + 1) * IB],
                                    start=(jt == 0), stop=(jt == NJT - 1),
                                )
                        # denominators live on psum partition DH; keep them
                        # there (no cross-partition DVE moves allowed).
                        den_t = sb.tile([DH + 1, 2 * IB], F32, tag="den", bufs=2)
                        for par in range(2):
                            h = 2 * pair + par
                            nc.vector.tensor_copy(
                                ot_sb[h][:, ib * IB:(ib + 1) * IB],
                                pv_ps[par][0:DH, :],
                            )
                            nc.vector.tensor_copy(
                                den_t[DH:DH + 1, par * IB:(par + 1) * IB],
                                pv_ps[par][DH:DH + 1, :],
                            )
                        nc.vector.reciprocal(den_t[DH:DH + 1, :],
                                             den_t[DH:DH + 1, :])
                        rrow = rden[pair * NIB + ib:pair * NIB + ib + 1, :]
                        nc.sync.dma_start(out=rrow, in_=den_t[DH:DH + 1, :])
                        for par in range(2):
                            h = 2 * pair + par
                            rb = sb.tile([DH, IB], F32, tag="rb", bufs=4)
                            src = rrow[0:1, par * IB:(par + 1) * IB]
                            bcast = bass.AP(
                                tensor=src.tensor, offset=src.offset,
                                ap=[[0, DH]] + [list(d) for d in src.ap[1:]],
                            )
                            nc.sync.dma_start(out=rb, in_=bcast)
                            nc.vector.tensor_mul(
                                ot_sb[h][:, ib * IB:(ib + 1) * IB],
                                ot_sb[h][:, ib * IB:(ib + 1) * IB],
                                rb,
                            )

            # ---------------- phase 3: output projection ----------------
            with tc.tile_pool(name="ps3", bufs=1, space="PSUM") as ps3:
                for dt4 in range(NDT):
                    for nb in range(NIB):
                        yp = ps3.tile([P, IB], F32, tag="y", bufs=4)
                        for h in range(HPC):
                            nc.tensor.matmul(
                                yp,
                                lhsT=wo_sb[h][:, dt4 * P:(dt4 + 1) * P],
                                rhs=ot_sb[h][:, nb * IB:(nb + 1) * IB],
                                start=(h == 0), stop=(h == HPC - 1),
                            )
                        yt_t = sb.tile([P, IB], F32, tag="yt", bufs=3)
                        nc.vector.tensor_scalar_add(yt_t, yp, bias_sb[dt4])
                        nc.sync.dma_start(
                            out=yT[dt4 * P:(dt4 + 1) * P, nb * IB:(nb + 1) * IB],
                            in_=yt_t,
                        )

    nc.finalize()
    return nc


_nc_cache = {}


def _get_program(n_reps):
    if n_reps not in _nc_cache:
        _nc_cache[n_reps] = build_program(n_reps)
    return _nc_cache[n_reps]


def make_in_maps(x, w_qkv, w_out, b_out):
    x = np.asarray(x, np.float32)
    w_qkv = np.asarray(w_qkv, np.float32)
    w_out = np.asarray(w_out, np.float32)
    b_out = np.asarray(b_out, np.float32)
    in_maps = []
    for core in range(N_CORES):
        b, hg = core // 2, core % 2
        s = 256 * hg
        wq = w_qkv[s:s + 256]
        wk = w_qkv[512 + s:512 + s + 256]
        wv_ = w_qkv[1024 + s:1024 + s + 256]
        in_maps.append({
            "xT": np.ascontiguousarray(x[b].T),
            "wqk": np.ascontiguousarray(np.concatenate([wq, wk], 0).T),
            "wv": np.ascontiguousarray(wv_.T),
            "wo": np.ascontiguousarray(w_out[:, s:s + 256].T),
            "bias": np.ascontiguousarray((b_out / 2).reshape(D, 1)),
        })
    return in_maps


def kernel(x, w_qkv, w_out, b_out):
    nc = _get_program(N_REPS)
    in_maps = make_in_maps(x, w_qkv, w_out, b_out)
    res = run_bass_kernel_spmd(nc, in_maps, list(range(N_CORES)))
    out = np.empty((B, N, D), np.float32)
    for b in range(B):
        out[b] = (res.results[2 * b]["yT"] + res.results[2 * b + 1]["yT"]).T
    return out


if __name__ == "__main__":
    nc = build_program(1)
    print("built OK; instructions:",
          sum(len(blk.instructions) for f in nc.m.functions for blk in f.blocks))

